# revision 38
# baseline (speedup 1.0000x reference)
"""GCN message-passing kernel for Trainium2, sharded over 8 NeuronCores.

Strategy (edge-cut partitioning per the sharding hint):
- Nodes renumbered so core c owns a contiguous block of 12544 (12500 real
  + 44 pad) destination nodes. Each edge is assigned to the core owning
  its destination; per core, edges are sorted by (source window of 32768
  rows, destination tile of 128 nodes) and padded so every (window, tile)
  segment is a whole number of 128-edge chunks, with identical chunk
  counts across cores (SPMD: one program, per-core data).
- Layer 1 aggregates RAW node features: since the linear commutes with
  the (linear) aggregation, per-edge messages are dis_s*x_s rows gathered
  straight from a host-prepared bf16 table (256B rows, 100% useful bytes,
  no device-side table build -> gathers start immediately). W1 + the
  deg-weighted bias correction c1[d]*b1 are applied per destination tile
  after accumulation, via a transpose + two chained PE matmuls.
- The scatter-add is a bf16 PE matmul with a per-chunk one-hot selection
  matrix (built on DVE from destination indices), accumulating in PSUM
  and bf16 SBUF accumulators; dis[col] is applied at tile finalize.
- Layer-2's table (dis_d * (W2 h1 + b2)) is built per-tile as soon as the
  tile's layer-1 accumulator finalizes (hidden behind layer-1 gathers)
  and exchanged with one bf16 AllGather; layer 2 then gathers it like
  layer 1. Graph pooling is likewise folded into layer-2 aggregation.
- The head (pool normalize + final linear + softmax) runs replicated
  after a small partial-sum collective (AllReduce, or AllGather + local
  sum via GNN_ARMODE=ag).
"""

import os
import sys

for _p in ("/opt/trn_rl_repo",):
    if _p not in sys.path:
        sys.path.insert(0, _p)

import numpy as np

N = 100000
E0 = 3200000
D_IN = 128
H1 = 32
H2 = 64
NCLS = 10
NG = 64
NCORES = 8
NLOC_REAL = 12500
NLOC = 12544          # per-core padded node count (98 * 128)
NP = NLOC * NCORES    # 100352 padded total
TLOC = NLOC // 128    # 98 dest tiles per core
WIN = 32768
NWIN = (NP + WIN - 1) // WIN  # 4
NTILES_G = NP // 128  # 784 global node tiles
CALL_CHUNKS = int(os.environ.get("GNN_CALLCHUNKS", "64"))  # chunks per dma_gather
NQ = int(os.environ.get("GNN_NQ", "4"))       # SWDGE queues
SP = os.environ.get("GNN_SP", "0") == "1"     # single_packet
GBUFS = int(os.environ.get("GNN_GBUFS", "4"))  # gather buffer depth


def _win_len(w):
    return min(WIN, NP - w * WIN)


def _host_prep(x, edge_index, batch):
    import ml_dtypes

    x = np.asarray(x, np.float32)
    ei = np.asarray(edge_index)
    batch = np.asarray(batch)

    # self loops participate in deg but are computed locally, not gathered
    row = ei[0].astype(np.int64)
    col = ei[1].astype(np.int64)
    deg = (np.bincount(row, minlength=N) + 1).astype(np.float32)  # +1 self loop

    # renumber: old g -> core c = g // 12500, new = c*NLOC + g % 12500
    def newid(g):
        return (g // NLOC_REAL) * NLOC + (g % NLOC_REAL)

    nrow = newid(row)
    ncol = newid(col)

    deg_new = np.ones(NP, np.float32)
    deg_new[newid(np.arange(N))] = deg
    dis_new = deg_new ** -0.5

    # layer-1 gather table: dis_s * x_s, node-major bf16 256B rows
    xdis = np.zeros((NP, D_IN), np.float32)
    xdis[newid(np.arange(N))] = x
    xdis *= dis_new[:, None]
    xdis16 = xdis.astype(ml_dtypes.bfloat16)

    # c1[d] = dis_d * (sum_{e->d} dis_s + dis_d): deg-weighted bias coef
    c1 = np.zeros(NP, np.float32)
    np.add.at(c1, ncol, dis_new[nrow])
    c1 += dis_new
    c1 *= dis_new

    cnt = np.bincount(np.asarray(batch, np.int64), minlength=NG).astype(np.float32)
    cnt = np.maximum(cnt, 1.0).reshape(NG, 1)

    # per-core edge structures
    ecore = ncol // NLOC
    percore = []
    for c in range(NCORES):
        m = ecore == c
        cr = nrow[m]
        cc = ncol[m] - c * NLOC
        w = (cr // WIN).astype(np.int64)
        t = (cc >> 7).astype(np.int64)
        dr = (cc & 127).astype(np.int64)
        order = np.lexsort((t, w))
        percore.append((cr[order], w[order], t[order], dr[order]))

    # segment chunk counts K[w][t], unified across cores
    K = np.zeros((NWIN, TLOC), np.int64)
    seg_counts = []
    for c in range(NCORES):
        cr, w, t, dr = percore[c]
        key = w * TLOC + t
        cnts = np.bincount(key, minlength=NWIN * TLOC).reshape(NWIN, TLOC)
        seg_counts.append(cnts)
        K = np.maximum(K, (cnts + 127) // 128)

    NCH = int(K.sum())
    EPAD = NCH * 128

    # per-core padded idx (window-relative) and dest_rel arrays
    idxmode = os.environ.get("GNN_IDXMODE", "real")
    idxw_list, drp_list = [], []
    for c in range(NCORES):
        cr, w, t, dr = percore[c]
        cnts = seg_counts[c]
        idx_flat = np.zeros(EPAD, np.int16)
        dr_flat = np.full(EPAD, -1.0, np.float32)
        pos_out = 0
        pos_in = 0
        for wi in range(NWIN):
            for ti in range(TLOC):
                n = int(cnts[wi, ti])
                kk = int(K[wi, ti]) * 128
                if kk == 0:
                    assert n == 0
                    continue
                seg_src = (cr[pos_in:pos_in + n] - wi * WIN).astype(np.int16)
                seg_dr = dr[pos_in:pos_in + n].astype(np.float32)
                if idxmode == "sortseg":
                    o = np.argsort(seg_src, kind="stable")
                    seg_src, seg_dr = seg_src[o], seg_dr[o]
                elif idxmode == "seq":
                    seg_src = ((pos_out + np.arange(n)) % _win_len(wi)).astype(np.int16)
                elif idxmode == "zero":
                    seg_src = np.zeros(n, np.int16)
                idx_flat[pos_out:pos_out + n] = seg_src
                dr_flat[pos_out:pos_out + n] = seg_dr
                pos_in += n
                pos_out += kk
        assert pos_in == len(cr) and pos_out == EPAD

        wrapped = idx_flat.reshape(EPAD // 16, 16).T.copy()  # [16, EPAD//16]
        idxw_list.append(np.tile(wrapped, (8, 1)))            # [128, EPAD//16]
        drp_list.append(np.ascontiguousarray(
            dr_flat.reshape(NCH, 128).T).astype(ml_dtypes.bfloat16))  # [128, NCH]

    # segments (w, t, q0, nk) and gather calls (w, q_start, nq, col0)
    segments = []
    q = 0
    for wi in range(NWIN):
        for ti in range(TLOC):
            nk = int(K[wi, ti])
            while nk > 0:  # split oversized segments to fit one gather call
                piece = min(nk, CALL_CHUNKS)
                segments.append((wi, ti, q, piece))
                q += piece
                nk -= piece
    assert q == NCH

    calls = []
    cur = None
    for (wi, ti, q0, nk) in segments:
        if cur is not None and cur[0] == wi and cur[2] + nk <= CALL_CHUNKS:
            cur[2] += nk
            cur[3].append((ti, q0, nk))
        else:
            if cur is not None:
                calls.append(cur)
            cur = [wi, q0, nk, [(ti, q0, nk)]]
    if cur is not None:
        calls.append(cur)

    # per-core local metadata
    degL_list, bo_list, xdl_list, c1_list = [], [], [], []
    batch64 = np.asarray(batch, np.int64)
    for c in range(NCORES):
        dl = deg_new[c * NLOC:(c + 1) * NLOC]
        degL_list.append(np.ascontiguousarray(dl.reshape(TLOC, 128).T))
        bo = np.full(NLOC, -1.0, np.float32)
        g0 = c * NLOC_REAL
        bo[:NLOC_REAL] = batch64[g0:g0 + NLOC_REAL].astype(np.float32)
        bo_list.append(np.ascontiguousarray(bo.reshape(TLOC, 128).T))
        # layer-1 accumulator init (self-loop term): [128, TLOC*128] bf16,
        # [p, t*128+f] = xdis[c*NLOC + t*128 + p, f]
        xl = xdis16[c * NLOC:(c + 1) * NLOC].reshape(TLOC, 128, D_IN)
        xdl_list.append(np.ascontiguousarray(
            xl.transpose(1, 0, 2).reshape(128, TLOC * D_IN)))
        c1_list.append(np.ascontiguousarray(
            c1[c * NLOC:(c + 1) * NLOC].reshape(1, NLOC)))

    return dict(xdis=xdis16, cnt=cnt, idxw=idxw_list, drp=drp_list,
                degL=degL_list, bo=bo_list, xdl=xdl_list, c1=c1_list,
                NCH=NCH, segments=segments, calls=calls)


def build_in_maps(prep, W1, b1, W2, b2, Wf, bf):
    import ml_dtypes

    ramp = np.tile(np.arange(128, dtype=np.float32), (128, 1))
    ident = np.eye(128, dtype=np.float32)
    common = dict(
        xdis=prep["xdis"], cnt=prep["cnt"],
        w1t=np.ascontiguousarray(
            np.asarray(W1, np.float32).T).astype(ml_dtypes.bfloat16),
        b1r=np.asarray(b1, np.float32).reshape(1, H1),
        w2t=np.ascontiguousarray(np.asarray(W2, np.float32).T),
        b2c=np.asarray(b2, np.float32).reshape(H2, 1),
        wft=np.ascontiguousarray(np.asarray(Wf, np.float32).T),
        bfc=np.asarray(bf, np.float32).reshape(NCLS, 1),
        ramp=ramp, ident=ident,
    )
    in_maps = []
    for c in range(NCORES):
        m = dict(common)
        m["idxw"] = prep["idxw"][c]
        m["drp"] = prep["drp"][c]
        m["degL"] = prep["degL"][c]
        m["bo"] = prep["bo"][c]
        m["xdl"] = prep["xdl"][c]
        m["c1"] = prep["c1"][c]
        in_maps.append(m)
    return in_maps


def _build_program(NCH, segments, calls):
    import concourse.bacc as bacc
    import concourse.mybir as mybir
    import concourse.tile as tile
    from concourse import library_config

    f32 = mybir.dt.float32
    bf16 = mybir.dt.bfloat16
    i16 = mybir.dt.int16
    AF = mybir.ActivationFunctionType
    OP = mybir.AluOpType

    nc = bacc.Bacc("TRN2", target_bir_lowering=False, debug=False,
                   num_devices=NCORES, num_swdge_queues=NQ)

    # I/O
    xdis = nc.dram_tensor("xdis", [NP, D_IN], bf16, kind="ExternalInput")
    xdl = nc.dram_tensor("xdl", [128, TLOC * D_IN], bf16, kind="ExternalInput")
    c1d = nc.dram_tensor("c1", [1, NLOC], f32, kind="ExternalInput")
    degL = nc.dram_tensor("degL", [128, TLOC], f32, kind="ExternalInput")
    idxw = nc.dram_tensor("idxw", [128, NCH * 8], i16, kind="ExternalInput")
    drp = nc.dram_tensor("drp", [128, NCH], bf16, kind="ExternalInput")
    bo = nc.dram_tensor("bo", [128, TLOC], f32, kind="ExternalInput")
    cntd = nc.dram_tensor("cnt", [NG, 1], f32, kind="ExternalInput")
    w1t = nc.dram_tensor("w1t", [D_IN, H1], bf16, kind="ExternalInput")
    b1rd = nc.dram_tensor("b1r", [1, H1], f32, kind="ExternalInput")
    w2t = nc.dram_tensor("w2t", [H1, H2], f32, kind="ExternalInput")
    b2c = nc.dram_tensor("b2c", [H2, 1], f32, kind="ExternalInput")
    wft = nc.dram_tensor("wft", [H2, NCLS], f32, kind="ExternalInput")
    bfc = nc.dram_tensor("bfc", [NCLS, 1], f32, kind="ExternalInput")
    rampd = nc.dram_tensor("ramp", [128, 128], f32, kind="ExternalInput")
    identd = nc.dram_tensor("ident", [128, 128], f32, kind="ExternalInput")
    y = nc.dram_tensor("y", [NG, NCLS], f32, kind="ExternalOutput")

    # internal DRAM
    tab2in = nc.dram_tensor("tab2in", [NLOC, 128], bf16)
    tab2 = nc.dram_tensor("tab2", [NP, 128], bf16, addr_space="Shared")
    cc2_in = nc.dram_tensor("cc2_in", [NG, H2], f32)
    cc2_out = nc.dram_tensor("cc2_out", [NG, H2], f32, addr_space="Shared")
    cc2_all = nc.dram_tensor("cc2_all", [NCORES * NG, H2], f32,
                             addr_space="Shared")

    tab2inv = tab2in.ap().rearrange("(a p) f -> p a f", p=128)  # [128, 98, 128]
    tab2copy = os.environ.get("GNN_TAB2COPY", "0") == "1"
    tab2b = nc.dram_tensor("tab2b", [NP, 128], bf16) if tab2copy else None
    fsrc_t = {"tab2": tab2b if tab2copy else tab2, "xdis": xdis}[
        os.environ.get("GNN_FSRC", "tab2")]
    src1 = [xdis.ap()[w * WIN:w * WIN + _win_len(w), :] for w in range(NWIN)]
    src2 = [fsrc_t.ap()[w * WIN:w * WIN + _win_len(w), :] for w in range(NWIN)]

    rg = [list(range(NCORES))]

    stages = os.environ.get("GNN_STAGES", "CDFG")
    aggmode = os.environ.get("GNN_AGGMODE", "full")
    armode = os.environ.get("GNN_ARMODE", "ar")

    with tile.TileContext(nc) as tc:
        nc.gpsimd.load_library(library_config.mlp)

        with tc.tile_pool(name="const", bufs=1) as cpool:
            ramp = cpool.tile([128, 128], f32)
            nc.sync.dma_start(out=ramp[:], in_=rampd[:])
            rampb = cpool.tile([128, 128], bf16)
            nc.vector.tensor_copy(rampb[:], ramp[:])
            ident = cpool.tile([128, 128], f32)
            nc.sync.dma_start(out=ident[:], in_=identd[:])
            identb = cpool.tile([128, 128], bf16)
            nc.vector.tensor_copy(identb[:], ident[:])
            drt = cpool.tile([128, NCH], bf16)
            nc.sync.dma_start(out=drt[:], in_=drp[:])
            w1s = cpool.tile([D_IN, H1], bf16)
            nc.sync.dma_start(out=w1s[:], in_=w1t[:])
            b1rs = cpool.tile([1, H1], f32)
            nc.sync.dma_start(out=b1rs[:], in_=b1rd[:])
            c1s = cpool.tile([1, NLOC], f32)
            nc.sync.dma_start(out=c1s[:], in_=c1d[:])
            w2s = cpool.tile([H1, H2], f32)
            nc.sync.dma_start(out=w2s[:], in_=w2t[:])
            b2s = cpool.tile([H2, 1], f32)
            nc.sync.dma_start(out=b2s[:], in_=b2c[:])
            wfs = cpool.tile([H2, NCLS], f32)
            nc.sync.dma_start(out=wfs[:], in_=wft[:])
            bfs = cpool.tile([NCLS, 1], f32)
            nc.sync.dma_start(out=bfs[:], in_=bfc[:])
            cnts = cpool.tile([NG, 1], f32)
            nc.sync.dma_start(out=cnts[:], in_=cntd[:])
            bos = cpool.tile([128, TLOC], f32)
            nc.sync.dma_start(out=bos[:], in_=bo[:])

            # dis local = degL ** -0.5
            dglt = cpool.tile([128, TLOC], f32)
            nc.sync.dma_start(out=dglt[:], in_=degL[:])
            disl = cpool.tile([128, TLOC], f32)
            nc.vector.reciprocal(disl[:], dglt[:])
            nc.scalar.activation(disl[:], disl[:], AF.Sqrt)

            acc1 = cpool.tile([128, TLOC * D_IN], bf16)
            acc2 = cpool.tile([128, TLOC * H2], bf16)

            # final segment-piece of each dest tile (for tile_cb interleave)
            _lastp = {}
            for (wi, ti, q0, nk) in segments:
                _lastp[ti] = (wi, q0)
            final_piece = {(v[0], k, v[1]) for k, v in _lastp.items()}

            def aggregate(wsrcs, Fl, acc, tile_cb=None, cb_pools=(),
                          ppbufs=4):
                qload = [0] * NQ  # least-loaded SWDGE queue assignment
                with tc.tile_pool(name="agg", bufs=GBUFS) as pool, \
                     tc.tile_pool(name="aggs", bufs=2) as spool, \
                     tc.tile_pool(name="aggi", bufs=4) as ipool, \
                     tc.tile_pool(name="aggp", bufs=ppbufs, space="PSUM") as pp:

                    def consume(wi, qs, nq, segs, S, gb):
                        for (ti, q0, nk) in segs:
                            ps = pp.tile([128, Fl], f32, tag="ps")
                            for k in range(nk):
                                slot = q0 + k - qs
                                lhs = rampb[:] if aggmode == "noS" \
                                    else S[:, slot, :]
                                nc.tensor.matmul(ps[:], lhs, gb[:, slot, 0:Fl],
                                                 start=(k == 0),
                                                 stop=(k == nk - 1),
                                                 skip_group_check=True)
                            nc.vector.tensor_tensor(
                                acc[:, ti * Fl:(ti + 1) * Fl],
                                acc[:, ti * Fl:(ti + 1) * Fl], ps[:], OP.add)
                            if tile_cb is not None and \
                                    (wi, ti, q0) in final_piece:
                                tile_cb(ti, *cb_pools)

                    # one-call software pipelining: emit call i+1's gather and
                    # one-hot build BEFORE call i's matmul/acc block, so the
                    # (gather-independent) S builds sit ahead of the stalling
                    # PSUM-drain adds in the in-order DVE queue.
                    pending = None
                    for ci, (wi, qs, nq, segs) in enumerate(calls):
                        it = ipool.tile([128, nq * 8], i16, tag="it")
                        # sync-ring HWDGE: the sync ring is nearly idle here,
                        # while the scalar/ACT engine runs the d_tile copies
                        nc.sync.dma_start(out=it[:],
                                          in_=idxw[:, qs * 8:(qs + nq) * 8])
                        gb = pool.tile([128, CALL_CHUNKS, 128], bf16, tag="gb")
                        qn = qload.index(min(qload))
                        qload[qn] += nq
                        nc.gpsimd.dma_gather(
                            gb[:, 0:nq, :], wsrcs[wi],
                            it[:], nq * 128, nq * 128, 128, single_packet=SP,
                            queue_num=qn)
                        if aggmode == "gather":
                            continue
                        S = None
                        if aggmode == "full":
                            # whole-call one-hot build: S[p, j, d] =
                            # (ramp[p, d] == dest_rel[p, qs+j]); the dest
                            # tile is encoded by the PSUM target, not S, so
                            # one ramp serves every segment of the call.
                            S = spool.tile([128, CALL_CHUNKS, 128], bf16,
                                           tag="S")
                            ramp_b = rampb[:].unsqueeze(1).broadcast_to(
                                [128, nq, 128])
                            dr_b = drt[:, qs:qs + nq].unsqueeze(2).broadcast_to(
                                [128, nq, 128])
                            nc.vector.tensor_tensor(S[:, 0:nq, :], ramp_b,
                                                    dr_b, OP.is_equal)
                        if pending is not None:
                            consume(*pending)
                        pending = (wi, qs, nq, segs, S, gb)
                    if pending is not None:
                        consume(*pending)

            def d_tile(t, pool, pp, pq):
                # finalize layer-1 tile t: apply W1 + bias, build tab2 rows
                # xw = bf16(dis_d * accX_tile)
                xw = pool.tile([128, 128], bf16, tag="xw")
                nc.vector.tensor_scalar(xw[:], acc1[:, t * 128:(t + 1) * 128],
                                        disl[:, t:t + 1], None, OP.mult)
                pT = pp.tile([128, 128], bf16, tag="pT")
                nc.tensor.transpose(pT[:], xw[:], identb[:])
                xTb = pool.tile([128, 128], bf16, tag="xTb")
                nc.scalar.activation(xTb[:], pT[:], AF.Copy)
                # h1T[h, d] = W1 @ (dis*accX)^T + outer(b1, c1)
                p1 = pq.tile([H1, 128], f32, tag="p1")
                nc.tensor.matmul(p1[:], w1s[:], xTb[:], start=True, stop=False)
                nc.tensor.matmul(p1[:], b1rs[0:1, :],
                                 c1s[0:1, t * 128:(t + 1) * 128],
                                 start=False, stop=True, skip_group_check=True)
                rT = pool.tile([H1, 128], f32, tag="rT")
                nc.scalar.activation(rT[:], p1[:], AF.Relu)
                # tab2 rows: dis_d * (W2 h1 + b2); also layer-2 self-loop init
                p2 = pp.tile([H2, 128], f32, tag="p2")
                nc.tensor.matmul(p2[:], w2s[:], rT[:], start=True, stop=True)
                hb2 = pool.tile([H2, 128], f32, tag="hb2")
                nc.vector.tensor_scalar(hb2[:], p2[:], b2s[:], None, OP.add)
                pj2 = pq.tile([128, H2], f32, tag="pj2")
                nc.tensor.transpose(pj2[:], hb2[:], ident[:H2, :H2])
                nc.vector.tensor_scalar(acc2[:, t * H2:(t + 1) * H2],
                                        pj2[:], disl[:, t:t + 1], None, OP.mult)
                nc.sync.dma_start(out=tab2inv[:, t, 0:H2],
                                  in_=acc2[:, t * H2:(t + 1) * H2])

            def tab2_allgather():
                nc.gpsimd.collective_compute(
                    "AllGather", mybir.AluOpType.bypass, replica_groups=rg,
                    ins=[tab2in.ap().opt()], outs=[tab2.ap().opt()])
                if tab2copy:
                    nc.sync.dma_start(out=tab2b.ap(), in_=tab2.ap())

            def g_tile(t, pool, pps, first, last):
                # pooled-sum contribution of dest tile t (per-graph one-hot)
                r2 = pool.tile([128, H2], f32, tag="r2")
                nc.scalar.activation(r2[:], acc2[:, t * H2:(t + 1) * H2],
                                     AF.Relu, scale=disl[:, t:t + 1])
                Sb = pool.tile([128, NG], f32, tag="Sb")
                nc.vector.tensor_scalar(Sb[:], ramp[:, 0:NG],
                                        bos[:, t:t + 1], None, OP.is_equal)
                nc.tensor.matmul(pps[:], Sb[:], r2[:],
                                 start=first, stop=last,
                                 skip_group_check=True)

            # callback emission order (order in which tiles finalize)
            cb_order = [ti for (wi, ti, q0, nk) in segments
                        if (wi, ti, q0) in final_piece]
            assert len(cb_order) == TLOC

            def pool_and_head(pps=None):
                with tc.tile_pool(name="hd", bufs=3) as pool, \
                     tc.tile_pool(name="hdp", bufs=1, space="PSUM") as pp:
                    if pps is None:
                        pps = pp.tile([NG, H2], f32, tag="pool")
                        for t in range(TLOC):
                            g_tile(t, pool, pps, t == 0, t == TLOC - 1)
                    pls = pool.tile([NG, H2], f32, tag="pls")
                    nc.scalar.activation(pls[:], pps[:], AF.Copy)
                    nc.sync.dma_start(out=cc2_in[:, :], in_=pls[:])
                    if armode == "none":
                        # timing probe only: skip the collective (wrong result)
                        psb = pool.tile([NG, H2], f32, tag="psb")
                        nc.sync.dma_start(out=psb[:], in_=cc2_in[:, :])
                    elif armode == "ag":
                        nc.gpsimd.collective_compute(
                            "AllGather", mybir.AluOpType.bypass,
                            replica_groups=rg,
                            ins=[cc2_in.ap().opt()], outs=[cc2_all.ap().opt()])
                        p8 = pool.tile([NG, NCORES, H2], f32, tag="p8")
                        nc.sync.dma_start(
                            out=p8[:],
                            in_=cc2_all.ap().rearrange("(c g) h -> g c h",
                                                       c=NCORES))
                        psb = pool.tile([NG, H2], f32, tag="psb")
                        nc.vector.tensor_tensor(psb[:], p8[:, 0, :],
                                                p8[:, 1, :], OP.add)
                        for c in range(2, NCORES):
                            nc.vector.tensor_tensor(
                                psb[:], psb[:], p8[:, c, :], OP.add)
                    else:
                        nc.gpsimd.collective_compute(
                            "AllReduce", OP.add, replica_groups=rg,
                            ins=[cc2_in.ap().opt()], outs=[cc2_out.ap().opt()])
                        psb = pool.tile([NG, H2], f32, tag="psb")
                        nc.sync.dma_start(out=psb[:], in_=cc2_out[:, :])
                    rc = pool.tile([NG, 1], f32, tag="rc")
                    nc.vector.reciprocal(rc[:], cnts[:])
                    mean = pool.tile([NG, H2], f32, tag="mean")
                    nc.vector.tensor_scalar(mean[:], psb[:], rc[:], None, OP.mult)
                    # transpose mean -> [H2, NG]
                    pmT = pp.tile([H2, NG], f32, tag="pmT")
                    nc.tensor.transpose(pmT[:], mean[:], ident[:NG, :NG])
                    meanT = pool.tile([H2, NG], f32, tag="meanT")
                    nc.scalar.activation(meanT[:], pmT[:], AF.Copy)
                    # logitsT [NCLS, NG]
                    plt = pp.tile([NCLS, NG], f32, tag="plt")
                    nc.tensor.matmul(plt[:], wfs[:], meanT[:], start=True, stop=True)
                    lts = pool.tile([NCLS, NG], f32, tag="lts")
                    nc.vector.tensor_scalar(lts[:], plt[:], bfs[:], None, OP.add)
                    # transpose -> [NG, NCLS]
                    plg = pp.tile([NG, NCLS], f32, tag="plg")
                    nc.tensor.transpose(plg[:], lts[:], ident[:NCLS, :NCLS])
                    lg = pool.tile([NG, NCLS], f32, tag="lg")
                    nc.scalar.activation(lg[:], plg[:], AF.Copy)
                    # softmax over free dim
                    mx = pool.tile([NG, 1], f32, tag="mx")
                    nc.vector.tensor_reduce(mx[:], lg[:], mybir.AxisListType.X,
                                            OP.max, negate=True)
                    ex = pool.tile([NG, NCLS], f32, tag="ex")
                    nc.scalar.activation(ex[:], lg[:], AF.Exp, bias=mx[:])
                    sm = pool.tile([NG, 1], f32, tag="sm")
                    nc.vector.tensor_reduce(sm[:], ex[:], mybir.AxisListType.X,
                                            OP.add)
                    rs = pool.tile([NG, 1], f32, tag="rs")
                    nc.vector.reciprocal(rs[:], sm[:])
                    yt = pool.tile([NG, NCLS], f32, tag="yt")
                    nc.vector.tensor_scalar(yt[:], ex[:], rs[:], None, OP.mult)
                    nc.sync.dma_start(out=y[:, :], in_=yt[:])

            di = os.environ.get("GNN_DI", "1") == "1"
            gi = os.environ.get("GNN_GI", "1") == "1"
            interleaved_d = di and "C" in stages and "D" in stages
            if "C" in stages:
                # layer-1 accumulator init: self-loop term dis_d * x_d
                nc.sync.dma_start(out=acc1[:], in_=xdl[:])
                if interleaved_d:
                    # fold the layer-2 table build into the aggregation: each
                    # dest tile's tab2 row block is computed as soon as its
                    # accumulator finalizes, hiding D behind the gathers.
                    with tc.tile_pool(name="rt", bufs=3) as dpool, \
                         tc.tile_pool(name="rtp", bufs=1, space="PSUM") as dpp, \
                         tc.tile_pool(name="rtq", bufs=1, space="PSUM") as dpq:
                        aggregate(src1, D_IN, acc1, tile_cb=d_tile,
                                  cb_pools=(dpool, dpp, dpq))
                    tab2_allgather()
                else:
                    aggregate(src1, D_IN, acc1)
            if "D" in stages and not interleaved_d:
                with tc.tile_pool(name="rt", bufs=3) as pool, \
                     tc.tile_pool(name="rtp", bufs=2, space="PSUM") as pp, \
                     tc.tile_pool(name="rtq", bufs=2, space="PSUM") as pq:
                    for t in range(TLOC):
                        d_tile(t, pool, pp, pq)
                tab2_allgather()
            interleaved_g = gi and "F" in stages and "G" in stages \
                and aggmode == "full"
            gpps = None
            if "F" in stages:
                if interleaved_g:
                    # fold the graph-pooling matmul into the aggregation:
                    # each dest tile is pooled as soon as it finalizes, so
                    # only the partial-sum collective + head remain after.
                    with tc.tile_pool(name="gp", bufs=1, space="PSUM") as gpp, \
                         tc.tile_pool(name="gs", bufs=2) as gspool:
                        gpps = gpp.tile([NG, H2], f32, tag="pool")
                        cb_first, cb_last = cb_order[0], cb_order[-1]

                        def _gcb(ti, pool, pps):
                            g_tile(ti, pool, pps, ti == cb_first, ti == cb_last)

                        aggregate(src2, H2, acc2, tile_cb=_gcb,
                                  cb_pools=(gspool, gpps),
                                  ppbufs=int(os.environ.get("GNN_FPP", "4")))
                        pool_and_head(gpps)
                else:
                    aggregate(src2, H2, acc2,
                              ppbufs=int(os.environ.get("GNN_FPP", "4")))
            if "G" in stages:
                if not interleaved_g:
                    pool_and_head()
            else:
                with tc.tile_pool(name="dbg", bufs=1) as dpool:
                    dt = dpool.tile([NG, NCLS], f32)
                    nc.vector.memset(dt[:], 0.0)
                    nc.sync.dma_start(out=y[:, :], in_=dt[:])

    nc.compile()
    return nc


def kernel(x, edge_index, batch, W1, b1, W2, b2, Wf, bf):
    from concourse.bass_utils import run_bass_kernel_spmd

    prep = _host_prep(x, edge_index, batch)
    nc = _build_program(prep["NCH"], prep["segments"], prep["calls"])
    in_maps = build_in_maps(prep, W1, b1, W2, b2, Wf, bf)
    res = run_bass_kernel_spmd(nc, in_maps, core_ids=list(range(NCORES)))
    return np.asarray(res.results[0]["y"], np.float32)


# revision 41
# speedup vs baseline: 1.0015x; 1.0015x over previous
"""GCN message-passing kernel for Trainium2, sharded over 8 NeuronCores.

Strategy (edge-cut partitioning per the sharding hint):
- Nodes renumbered so core c owns a contiguous block of 12544 (12500 real
  + 44 pad) destination nodes. Each edge is assigned to the core owning
  its destination; per core, edges are sorted by (source window of 32768
  rows, destination tile of 128 nodes) and padded so every (window, tile)
  segment is a whole number of 128-edge chunks, with identical chunk
  counts across cores (SPMD: one program, per-core data).
- Layer 1 aggregates RAW node features: since the linear commutes with
  the (linear) aggregation, per-edge messages are dis_s*x_s rows gathered
  straight from a host-prepared bf16 table (256B rows, 100% useful bytes,
  no device-side table build -> gathers start immediately). W1 + the
  deg-weighted bias correction c1[d]*b1 are applied per destination tile
  after accumulation, via a transpose + two chained PE matmuls.
- The scatter-add is a bf16 PE matmul with a per-chunk one-hot selection
  matrix (built on DVE from destination indices), accumulating in PSUM
  and bf16 SBUF accumulators; dis[col] is applied at tile finalize.
- Layer-2's table (dis_d * (W2 h1 + b2)) is built per-tile as soon as the
  tile's layer-1 accumulator finalizes (hidden behind layer-1 gathers)
  and exchanged with one bf16 AllGather; layer 2 then gathers it like
  layer 1. Graph pooling is likewise folded into layer-2 aggregation.
- The head (pool normalize + final linear + softmax) runs replicated
  after a small partial-sum collective (AllReduce, or AllGather + local
  sum via GNN_ARMODE=ag).
"""

import os
import sys

for _p in ("/opt/trn_rl_repo",):
    if _p not in sys.path:
        sys.path.insert(0, _p)

import numpy as np

N = 100000
E0 = 3200000
D_IN = 128
H1 = 32
H2 = 64
NCLS = 10
NG = 64
NCORES = 8
NLOC_REAL = 12500
NLOC = 12544          # per-core padded node count (98 * 128)
NP = NLOC * NCORES    # 100352 padded total
TLOC = NLOC // 128    # 98 dest tiles per core
WIN = 32768
NWIN = (NP + WIN - 1) // WIN  # 4
NTILES_G = NP // 128  # 784 global node tiles
CALL_CHUNKS = int(os.environ.get("GNN_CALLCHUNKS", "64"))  # chunks per dma_gather
NQ = int(os.environ.get("GNN_NQ", "4"))       # SWDGE queues
SP = os.environ.get("GNN_SP", "0") == "1"     # single_packet
GBUFS = int(os.environ.get("GNN_GBUFS", "4"))  # gather buffer depth


def _win_len(w):
    return min(WIN, NP - w * WIN)


def _host_prep(x, edge_index, batch):
    import ml_dtypes

    x = np.asarray(x, np.float32)
    ei = np.asarray(edge_index)
    batch = np.asarray(batch)

    # self loops participate in deg but are computed locally, not gathered
    row = ei[0].astype(np.int64)
    col = ei[1].astype(np.int64)
    deg = (np.bincount(row, minlength=N) + 1).astype(np.float32)  # +1 self loop

    # renumber: old g -> core c = g // 12500, new = c*NLOC + g % 12500
    def newid(g):
        return (g // NLOC_REAL) * NLOC + (g % NLOC_REAL)

    nrow = newid(row)
    ncol = newid(col)

    deg_new = np.ones(NP, np.float32)
    deg_new[newid(np.arange(N))] = deg
    dis_new = deg_new ** -0.5

    # layer-1 gather table: dis_s * x_s, node-major bf16 256B rows
    xdis = np.zeros((NP, D_IN), np.float32)
    xdis[newid(np.arange(N))] = x
    xdis *= dis_new[:, None]
    xdis16 = xdis.astype(ml_dtypes.bfloat16)

    # c1[d] = dis_d * (sum_{e->d} dis_s + dis_d): deg-weighted bias coef
    c1 = np.zeros(NP, np.float32)
    np.add.at(c1, ncol, dis_new[nrow])
    c1 += dis_new
    c1 *= dis_new

    cnt = np.bincount(np.asarray(batch, np.int64), minlength=NG).astype(np.float32)
    cnt = np.maximum(cnt, 1.0).reshape(NG, 1)

    # per-core edge structures
    ecore = ncol // NLOC
    percore = []
    for c in range(NCORES):
        m = ecore == c
        cr = nrow[m]
        cc = ncol[m] - c * NLOC
        w = (cr // WIN).astype(np.int64)
        t = (cc >> 7).astype(np.int64)
        dr = (cc & 127).astype(np.int64)
        order = np.lexsort((t, w))
        percore.append((cr[order], w[order], t[order], dr[order]))

    # segment chunk counts K[w][t], unified across cores
    K = np.zeros((NWIN, TLOC), np.int64)
    seg_counts = []
    for c in range(NCORES):
        cr, w, t, dr = percore[c]
        key = w * TLOC + t
        cnts = np.bincount(key, minlength=NWIN * TLOC).reshape(NWIN, TLOC)
        seg_counts.append(cnts)
        K = np.maximum(K, (cnts + 127) // 128)

    NCH = int(K.sum())
    EPAD = NCH * 128

    # per-core padded idx (window-relative) and dest_rel arrays
    idxmode = os.environ.get("GNN_IDXMODE", "real")
    idxw_list, drp_list = [], []
    for c in range(NCORES):
        cr, w, t, dr = percore[c]
        cnts = seg_counts[c]
        idx_flat = np.zeros(EPAD, np.int16)
        dr_flat = np.full(EPAD, -1.0, np.float32)
        pos_out = 0
        pos_in = 0
        for wi in range(NWIN):
            for ti in range(TLOC):
                n = int(cnts[wi, ti])
                kk = int(K[wi, ti]) * 128
                if kk == 0:
                    assert n == 0
                    continue
                seg_src = (cr[pos_in:pos_in + n] - wi * WIN).astype(np.int16)
                seg_dr = dr[pos_in:pos_in + n].astype(np.float32)
                if idxmode == "sortseg":
                    o = np.argsort(seg_src, kind="stable")
                    seg_src, seg_dr = seg_src[o], seg_dr[o]
                elif idxmode == "seq":
                    seg_src = ((pos_out + np.arange(n)) % _win_len(wi)).astype(np.int16)
                elif idxmode == "zero":
                    seg_src = np.zeros(n, np.int16)
                idx_flat[pos_out:pos_out + n] = seg_src
                dr_flat[pos_out:pos_out + n] = seg_dr
                pos_in += n
                pos_out += kk
        assert pos_in == len(cr) and pos_out == EPAD

        wrapped = idx_flat.reshape(EPAD // 16, 16).T.copy()  # [16, EPAD//16]
        idxw_list.append(np.tile(wrapped, (8, 1)))            # [128, EPAD//16]
        drp_list.append(np.ascontiguousarray(
            dr_flat.reshape(NCH, 128).T).astype(ml_dtypes.bfloat16))  # [128, NCH]

    # segments (w, t, q0, nk) and gather calls (w, q_start, nq, col0)
    segments = []
    q = 0
    for wi in range(NWIN):
        for ti in range(TLOC):
            nk = int(K[wi, ti])
            while nk > 0:  # split oversized segments to fit one gather call
                piece = min(nk, CALL_CHUNKS)
                segments.append((wi, ti, q, piece))
                q += piece
                nk -= piece
    assert q == NCH

    calls = []
    cur = None
    for (wi, ti, q0, nk) in segments:
        if cur is not None and cur[0] == wi and cur[2] + nk <= CALL_CHUNKS:
            cur[2] += nk
            cur[3].append((ti, q0, nk))
        else:
            if cur is not None:
                calls.append(cur)
            cur = [wi, q0, nk, [(ti, q0, nk)]]
    if cur is not None:
        calls.append(cur)

    # per-core local metadata
    degL_list, bo_list, xdl_list, c1_list = [], [], [], []
    batch64 = np.asarray(batch, np.int64)
    for c in range(NCORES):
        dl = deg_new[c * NLOC:(c + 1) * NLOC]
        degL_list.append(np.ascontiguousarray(dl.reshape(TLOC, 128).T))
        bo = np.full(NLOC, -1.0, np.float32)
        g0 = c * NLOC_REAL
        bo[:NLOC_REAL] = batch64[g0:g0 + NLOC_REAL].astype(np.float32)
        bo_list.append(np.ascontiguousarray(bo.reshape(TLOC, 128).T))
        # layer-1 accumulator init (self-loop term): [128, TLOC*128] bf16,
        # [p, t*128+f] = xdis[c*NLOC + t*128 + p, f]
        xl = xdis16[c * NLOC:(c + 1) * NLOC].reshape(TLOC, 128, D_IN)
        xdl_list.append(np.ascontiguousarray(
            xl.transpose(1, 0, 2).reshape(128, TLOC * D_IN)))
        c1_list.append(np.ascontiguousarray(
            c1[c * NLOC:(c + 1) * NLOC].reshape(1, NLOC)))

    return dict(xdis=xdis16, cnt=cnt, idxw=idxw_list, drp=drp_list,
                degL=degL_list, bo=bo_list, xdl=xdl_list, c1=c1_list,
                NCH=NCH, segments=segments, calls=calls)


def build_in_maps(prep, W1, b1, W2, b2, Wf, bf):
    import ml_dtypes

    ramp = np.tile(np.arange(128, dtype=np.float32), (128, 1))
    ident = np.eye(128, dtype=np.float32)
    common = dict(
        xdis=prep["xdis"], cnt=prep["cnt"],
        w1t=np.ascontiguousarray(
            np.asarray(W1, np.float32).T).astype(ml_dtypes.bfloat16),
        b1r=np.asarray(b1, np.float32).reshape(1, H1),
        w2t=np.ascontiguousarray(np.asarray(W2, np.float32).T),
        b2c=np.asarray(b2, np.float32).reshape(H2, 1),
        wft=np.ascontiguousarray(np.asarray(Wf, np.float32).T),
        bfc=np.asarray(bf, np.float32).reshape(NCLS, 1),
        ramp=ramp, ident=ident,
    )
    in_maps = []
    for c in range(NCORES):
        m = dict(common)
        m["idxw"] = prep["idxw"][c]
        m["drp"] = prep["drp"][c]
        m["degL"] = prep["degL"][c]
        m["bo"] = prep["bo"][c]
        m["xdl"] = prep["xdl"][c]
        m["c1"] = prep["c1"][c]
        in_maps.append(m)
    return in_maps


def _build_program(NCH, segments, calls):
    import concourse.bacc as bacc
    import concourse.mybir as mybir
    import concourse.tile as tile
    from concourse import library_config

    f32 = mybir.dt.float32
    bf16 = mybir.dt.bfloat16
    i16 = mybir.dt.int16
    AF = mybir.ActivationFunctionType
    OP = mybir.AluOpType

    nc = bacc.Bacc("TRN2", target_bir_lowering=False, debug=False,
                   num_devices=NCORES, num_swdge_queues=NQ)

    # I/O
    xdis = nc.dram_tensor("xdis", [NP, D_IN], bf16, kind="ExternalInput")
    xdl = nc.dram_tensor("xdl", [128, TLOC * D_IN], bf16, kind="ExternalInput")
    c1d = nc.dram_tensor("c1", [1, NLOC], f32, kind="ExternalInput")
    degL = nc.dram_tensor("degL", [128, TLOC], f32, kind="ExternalInput")
    idxw = nc.dram_tensor("idxw", [128, NCH * 8], i16, kind="ExternalInput")
    drp = nc.dram_tensor("drp", [128, NCH], bf16, kind="ExternalInput")
    bo = nc.dram_tensor("bo", [128, TLOC], f32, kind="ExternalInput")
    cntd = nc.dram_tensor("cnt", [NG, 1], f32, kind="ExternalInput")
    w1t = nc.dram_tensor("w1t", [D_IN, H1], bf16, kind="ExternalInput")
    b1rd = nc.dram_tensor("b1r", [1, H1], f32, kind="ExternalInput")
    w2t = nc.dram_tensor("w2t", [H1, H2], f32, kind="ExternalInput")
    b2c = nc.dram_tensor("b2c", [H2, 1], f32, kind="ExternalInput")
    wft = nc.dram_tensor("wft", [H2, NCLS], f32, kind="ExternalInput")
    bfc = nc.dram_tensor("bfc", [NCLS, 1], f32, kind="ExternalInput")
    rampd = nc.dram_tensor("ramp", [128, 128], f32, kind="ExternalInput")
    identd = nc.dram_tensor("ident", [128, 128], f32, kind="ExternalInput")
    y = nc.dram_tensor("y", [NG, NCLS], f32, kind="ExternalOutput")

    # internal DRAM
    tab2in = nc.dram_tensor("tab2in", [NLOC, 128], bf16)
    tab2 = nc.dram_tensor("tab2", [NP, 128], bf16, addr_space="Shared")
    cc2_in = nc.dram_tensor("cc2_in", [NG, H2], f32)
    cc2_out = nc.dram_tensor("cc2_out", [NG, H2], f32, addr_space="Shared")
    cc2_all = nc.dram_tensor("cc2_all", [NCORES * NG, H2], f32,
                             addr_space="Shared")

    tab2inv = tab2in.ap().rearrange("(a p) f -> p a f", p=128)  # [128, 98, 128]
    tab2copy = os.environ.get("GNN_TAB2COPY", "0") == "1"
    tab2b = nc.dram_tensor("tab2b", [NP, 128], bf16) if tab2copy else None
    fsrc_t = {"tab2": tab2b if tab2copy else tab2, "xdis": xdis}[
        os.environ.get("GNN_FSRC", "tab2")]
    src1 = [xdis.ap()[w * WIN:w * WIN + _win_len(w), :] for w in range(NWIN)]
    src2 = [fsrc_t.ap()[w * WIN:w * WIN + _win_len(w), :] for w in range(NWIN)]

    rg = [list(range(NCORES))]

    stages = os.environ.get("GNN_STAGES", "CDFG")
    aggmode = os.environ.get("GNN_AGGMODE", "full")
    armode = os.environ.get("GNN_ARMODE", "ar")

    with tile.TileContext(nc) as tc:
        nc.gpsimd.load_library(library_config.mlp)

        with tc.tile_pool(name="const", bufs=1) as cpool:
            ramp = cpool.tile([128, 128], f32)
            nc.sync.dma_start(out=ramp[:], in_=rampd[:])
            rampb = cpool.tile([128, 128], bf16)
            nc.vector.tensor_copy(rampb[:], ramp[:])
            ident = cpool.tile([128, 128], f32)
            nc.sync.dma_start(out=ident[:], in_=identd[:])
            identb = cpool.tile([128, 128], bf16)
            nc.vector.tensor_copy(identb[:], ident[:])
            drt = cpool.tile([128, NCH], bf16)
            nc.sync.dma_start(out=drt[:], in_=drp[:])
            w1s = cpool.tile([D_IN, H1], bf16)
            nc.sync.dma_start(out=w1s[:], in_=w1t[:])
            b1rs = cpool.tile([1, H1], f32)
            nc.sync.dma_start(out=b1rs[:], in_=b1rd[:])
            c1s = cpool.tile([1, NLOC], f32)
            nc.sync.dma_start(out=c1s[:], in_=c1d[:])
            w2s = cpool.tile([H1, H2], f32)
            nc.sync.dma_start(out=w2s[:], in_=w2t[:])
            b2s = cpool.tile([H2, 1], f32)
            nc.sync.dma_start(out=b2s[:], in_=b2c[:])
            wfs = cpool.tile([H2, NCLS], f32)
            nc.sync.dma_start(out=wfs[:], in_=wft[:])
            bfs = cpool.tile([NCLS, 1], f32)
            nc.sync.dma_start(out=bfs[:], in_=bfc[:])
            cnts = cpool.tile([NG, 1], f32)
            nc.sync.dma_start(out=cnts[:], in_=cntd[:])
            bos = cpool.tile([128, TLOC], f32)
            nc.sync.dma_start(out=bos[:], in_=bo[:])

            # dis local = degL ** -0.5
            dglt = cpool.tile([128, TLOC], f32)
            nc.sync.dma_start(out=dglt[:], in_=degL[:])
            disl = cpool.tile([128, TLOC], f32)
            nc.vector.reciprocal(disl[:], dglt[:])
            nc.scalar.activation(disl[:], disl[:], AF.Sqrt)

            acc1 = cpool.tile([128, TLOC * D_IN], bf16)
            acc2 = cpool.tile([128, TLOC * H2], bf16)

            # final segment-piece of each dest tile (for tile_cb interleave)
            _lastp = {}
            for (wi, ti, q0, nk) in segments:
                _lastp[ti] = (wi, q0)
            final_piece = {(v[0], k, v[1]) for k, v in _lastp.items()}

            def aggregate(wsrcs, Fl, acc, tile_cb=None, cb_pools=(),
                          ppbufs=4):
                qload = [0] * NQ  # least-loaded SWDGE queue assignment
                HCH = CALL_CHUNKS // 2
                with tc.tile_pool(name="agg", bufs=GBUFS) as pool, \
                     tc.tile_pool(name="aggs", bufs=2) as spool, \
                     tc.tile_pool(name="aggi", bufs=4) as ipool, \
                     tc.tile_pool(name="aggp", bufs=ppbufs, space="PSUM") as pp:

                    def consume(wi, qs, nq, segs, S, gb):
                        for (ti, q0, nk) in segs:
                            ps = pp.tile([128, Fl], f32, tag="ps")
                            for k in range(nk):
                                slot = q0 + k - qs
                                lhs = rampb[:] if aggmode == "noS" \
                                    else S[slot // HCH][:, slot % HCH, :]
                                nc.tensor.matmul(ps[:], lhs, gb[:, slot, 0:Fl],
                                                 start=(k == 0),
                                                 stop=(k == nk - 1),
                                                 skip_group_check=True)
                            nc.vector.tensor_tensor(
                                acc[:, ti * Fl:(ti + 1) * Fl],
                                acc[:, ti * Fl:(ti + 1) * Fl], ps[:], OP.add)
                            if tile_cb is not None and \
                                    (wi, ti, q0) in final_piece:
                                tile_cb(ti, *cb_pools)

                    # one-call software pipelining: emit call i+1's gather and
                    # one-hot build BEFORE call i's matmul/acc block, so the
                    # (gather-independent) S builds sit ahead of the stalling
                    # PSUM-drain adds in the in-order DVE queue.
                    pending = None
                    for ci, (wi, qs, nq, segs) in enumerate(calls):
                        it = ipool.tile([128, nq * 8], i16, tag="it")
                        # sync-ring HWDGE: the sync ring is nearly idle here,
                        # while the scalar/ACT engine runs the d_tile copies
                        nc.sync.dma_start(out=it[:],
                                          in_=idxw[:, qs * 8:(qs + nq) * 8])
                        gb = pool.tile([128, CALL_CHUNKS, 128], bf16, tag="gb")
                        qn = qload.index(min(qload))
                        qload[qn] += nq
                        nc.gpsimd.dma_gather(
                            gb[:, 0:nq, :], wsrcs[wi],
                            it[:], nq * 128, nq * 128, 128, single_packet=SP,
                            queue_num=qn)
                        if aggmode == "gather":
                            continue
                        S = None
                        if aggmode == "full":
                            # whole-call one-hot build (two half-call tiles):
                            # S[p, j, d] = (ramp[p, d] == dest_rel[p, qs+j]);
                            # the dest tile is encoded by the PSUM target,
                            # not S, so one ramp serves every segment.
                            S = []
                            for h in range(2):
                                hn = min(nq - h * HCH, HCH)
                                Sh = spool.tile([128, HCH, 128], bf16,
                                                tag=f"S{h}")
                                S.append(Sh)
                                if hn <= 0:
                                    continue
                                ramp_b = rampb[:].unsqueeze(1).broadcast_to(
                                    [128, hn, 128])
                                dr_b = drt[:, qs + h * HCH:qs + h * HCH + hn] \
                                    .unsqueeze(2).broadcast_to([128, hn, 128])
                                nc.vector.tensor_tensor(Sh[:, 0:hn, :],
                                                        ramp_b, dr_b,
                                                        OP.is_equal)
                        if pending is not None:
                            consume(*pending)
                        pending = (wi, qs, nq, segs, S, gb)
                    if pending is not None:
                        consume(*pending)

            def d_tile(t, pool, pp, pq):
                # finalize layer-1 tile t: apply W1 + bias, build tab2 rows
                # xw = bf16(dis_d * accX_tile)
                xw = pool.tile([128, 128], bf16, tag="xw")
                nc.vector.tensor_scalar(xw[:], acc1[:, t * 128:(t + 1) * 128],
                                        disl[:, t:t + 1], None, OP.mult)
                pT = pp.tile([128, 128], bf16, tag="pT")
                nc.tensor.transpose(pT[:], xw[:], identb[:])
                xTb = pool.tile([128, 128], bf16, tag="xTb")
                nc.scalar.activation(xTb[:], pT[:], AF.Copy)
                # h1T[h, d] = W1 @ (dis*accX)^T + outer(b1, c1)
                p1 = pq.tile([H1, 128], f32, tag="p1")
                nc.tensor.matmul(p1[:], w1s[:], xTb[:], start=True, stop=False)
                nc.tensor.matmul(p1[:], b1rs[0:1, :],
                                 c1s[0:1, t * 128:(t + 1) * 128],
                                 start=False, stop=True, skip_group_check=True)
                rT = pool.tile([H1, 128], f32, tag="rT")
                nc.scalar.activation(rT[:], p1[:], AF.Relu)
                # tab2 rows: dis_d * (W2 h1 + b2); also layer-2 self-loop init
                p2 = pp.tile([H2, 128], f32, tag="p2")
                nc.tensor.matmul(p2[:], w2s[:], rT[:], start=True, stop=True)
                hb2 = pool.tile([H2, 128], f32, tag="hb2")
                nc.vector.tensor_scalar(hb2[:], p2[:], b2s[:], None, OP.add)
                pj2 = pq.tile([128, H2], f32, tag="pj2")
                nc.tensor.transpose(pj2[:], hb2[:], ident[:H2, :H2])
                nc.vector.tensor_scalar(acc2[:, t * H2:(t + 1) * H2],
                                        pj2[:], disl[:, t:t + 1], None, OP.mult)
                nc.sync.dma_start(out=tab2inv[:, t, 0:H2],
                                  in_=acc2[:, t * H2:(t + 1) * H2])

            def tab2_allgather():
                nc.gpsimd.collective_compute(
                    "AllGather", mybir.AluOpType.bypass, replica_groups=rg,
                    ins=[tab2in.ap().opt()], outs=[tab2.ap().opt()])
                if tab2copy:
                    nc.sync.dma_start(out=tab2b.ap(), in_=tab2.ap())

            def g_tile(t, pool, pps, first, last):
                # pooled-sum contribution of dest tile t (per-graph one-hot)
                r2 = pool.tile([128, H2], f32, tag="r2")
                nc.scalar.activation(r2[:], acc2[:, t * H2:(t + 1) * H2],
                                     AF.Relu, scale=disl[:, t:t + 1])
                Sb = pool.tile([128, NG], f32, tag="Sb")
                nc.vector.tensor_scalar(Sb[:], ramp[:, 0:NG],
                                        bos[:, t:t + 1], None, OP.is_equal)
                nc.tensor.matmul(pps[:], Sb[:], r2[:],
                                 start=first, stop=last,
                                 skip_group_check=True)

            # callback emission order (order in which tiles finalize)
            cb_order = [ti for (wi, ti, q0, nk) in segments
                        if (wi, ti, q0) in final_piece]
            assert len(cb_order) == TLOC

            def pool_and_head(pps=None):
                with tc.tile_pool(name="hd", bufs=3) as pool, \
                     tc.tile_pool(name="hdp", bufs=1, space="PSUM") as pp:
                    if pps is None:
                        pps = pp.tile([NG, H2], f32, tag="pool")
                        for t in range(TLOC):
                            g_tile(t, pool, pps, t == 0, t == TLOC - 1)
                    pls = pool.tile([NG, H2], f32, tag="pls")
                    nc.scalar.activation(pls[:], pps[:], AF.Copy)
                    nc.sync.dma_start(out=cc2_in[:, :], in_=pls[:])
                    if armode == "none":
                        # timing probe only: skip the collective (wrong result)
                        psb = pool.tile([NG, H2], f32, tag="psb")
                        nc.sync.dma_start(out=psb[:], in_=cc2_in[:, :])
                    elif armode == "ag":
                        nc.gpsimd.collective_compute(
                            "AllGather", mybir.AluOpType.bypass,
                            replica_groups=rg,
                            ins=[cc2_in.ap().opt()], outs=[cc2_all.ap().opt()])
                        p8 = pool.tile([NG, NCORES, H2], f32, tag="p8")
                        nc.sync.dma_start(
                            out=p8[:],
                            in_=cc2_all.ap().rearrange("(c g) h -> g c h",
                                                       c=NCORES))
                        psb = pool.tile([NG, H2], f32, tag="psb")
                        nc.vector.tensor_tensor(psb[:], p8[:, 0, :],
                                                p8[:, 1, :], OP.add)
                        for c in range(2, NCORES):
                            nc.vector.tensor_tensor(
                                psb[:], psb[:], p8[:, c, :], OP.add)
                    else:
                        nc.gpsimd.collective_compute(
                            "AllReduce", OP.add, replica_groups=rg,
                            ins=[cc2_in.ap().opt()], outs=[cc2_out.ap().opt()])
                        psb = pool.tile([NG, H2], f32, tag="psb")
                        nc.sync.dma_start(out=psb[:], in_=cc2_out[:, :])
                    rc = pool.tile([NG, 1], f32, tag="rc")
                    nc.vector.reciprocal(rc[:], cnts[:])
                    mean = pool.tile([NG, H2], f32, tag="mean")
                    nc.vector.tensor_scalar(mean[:], psb[:], rc[:], None, OP.mult)
                    # transpose mean -> [H2, NG]
                    pmT = pp.tile([H2, NG], f32, tag="pmT")
                    nc.tensor.transpose(pmT[:], mean[:], ident[:NG, :NG])
                    meanT = pool.tile([H2, NG], f32, tag="meanT")
                    nc.scalar.activation(meanT[:], pmT[:], AF.Copy)
                    # logitsT [NCLS, NG]
                    plt = pp.tile([NCLS, NG], f32, tag="plt")
                    nc.tensor.matmul(plt[:], wfs[:], meanT[:], start=True, stop=True)
                    lts = pool.tile([NCLS, NG], f32, tag="lts")
                    nc.vector.tensor_scalar(lts[:], plt[:], bfs[:], None, OP.add)
                    # transpose -> [NG, NCLS]
                    plg = pp.tile([NG, NCLS], f32, tag="plg")
                    nc.tensor.transpose(plg[:], lts[:], ident[:NCLS, :NCLS])
                    lg = pool.tile([NG, NCLS], f32, tag="lg")
                    nc.scalar.activation(lg[:], plg[:], AF.Copy)
                    # softmax over free dim
                    mx = pool.tile([NG, 1], f32, tag="mx")
                    nc.vector.tensor_reduce(mx[:], lg[:], mybir.AxisListType.X,
                                            OP.max, negate=True)
                    ex = pool.tile([NG, NCLS], f32, tag="ex")
                    nc.scalar.activation(ex[:], lg[:], AF.Exp, bias=mx[:])
                    sm = pool.tile([NG, 1], f32, tag="sm")
                    nc.vector.tensor_reduce(sm[:], ex[:], mybir.AxisListType.X,
                                            OP.add)
                    rs = pool.tile([NG, 1], f32, tag="rs")
                    nc.vector.reciprocal(rs[:], sm[:])
                    yt = pool.tile([NG, NCLS], f32, tag="yt")
                    nc.vector.tensor_scalar(yt[:], ex[:], rs[:], None, OP.mult)
                    nc.sync.dma_start(out=y[:, :], in_=yt[:])

            di = os.environ.get("GNN_DI", "1") == "1"
            gi = os.environ.get("GNN_GI", "1") == "1"
            interleaved_d = di and "C" in stages and "D" in stages
            if "C" in stages:
                # layer-1 accumulator init: self-loop term dis_d * x_d
                nc.sync.dma_start(out=acc1[:], in_=xdl[:])
                if interleaved_d:
                    # fold the layer-2 table build into the aggregation: each
                    # dest tile's tab2 row block is computed as soon as its
                    # accumulator finalizes, hiding D behind the gathers.
                    with tc.tile_pool(name="rt", bufs=3) as dpool, \
                         tc.tile_pool(name="rtp", bufs=1, space="PSUM") as dpp, \
                         tc.tile_pool(name="rtq", bufs=1, space="PSUM") as dpq:
                        aggregate(src1, D_IN, acc1, tile_cb=d_tile,
                                  cb_pools=(dpool, dpp, dpq))
                    tab2_allgather()
                else:
                    aggregate(src1, D_IN, acc1)
            if "D" in stages and not interleaved_d:
                with tc.tile_pool(name="rt", bufs=3) as pool, \
                     tc.tile_pool(name="rtp", bufs=2, space="PSUM") as pp, \
                     tc.tile_pool(name="rtq", bufs=2, space="PSUM") as pq:
                    for t in range(TLOC):
                        d_tile(t, pool, pp, pq)
                tab2_allgather()
            interleaved_g = gi and "F" in stages and "G" in stages \
                and aggmode == "full"
            gpps = None
            if "F" in stages:
                if interleaved_g:
                    # fold the graph-pooling matmul into the aggregation:
                    # each dest tile is pooled as soon as it finalizes, so
                    # only the partial-sum collective + head remain after.
                    with tc.tile_pool(name="gp", bufs=1, space="PSUM") as gpp, \
                         tc.tile_pool(name="gs", bufs=2) as gspool:
                        gpps = gpp.tile([NG, H2], f32, tag="pool")
                        cb_first, cb_last = cb_order[0], cb_order[-1]

                        def _gcb(ti, pool, pps):
                            g_tile(ti, pool, pps, ti == cb_first, ti == cb_last)

                        aggregate(src2, H2, acc2, tile_cb=_gcb,
                                  cb_pools=(gspool, gpps),
                                  ppbufs=int(os.environ.get("GNN_FPP", "4")))
                        pool_and_head(gpps)
                else:
                    aggregate(src2, H2, acc2,
                              ppbufs=int(os.environ.get("GNN_FPP", "4")))
            if "G" in stages:
                if not interleaved_g:
                    pool_and_head()
            else:
                with tc.tile_pool(name="dbg", bufs=1) as dpool:
                    dt = dpool.tile([NG, NCLS], f32)
                    nc.vector.memset(dt[:], 0.0)
                    nc.sync.dma_start(out=y[:, :], in_=dt[:])

    nc.compile()
    return nc


def kernel(x, edge_index, batch, W1, b1, W2, b2, Wf, bf):
    from concourse.bass_utils import run_bass_kernel_spmd

    prep = _host_prep(x, edge_index, batch)
    nc = _build_program(prep["NCH"], prep["segments"], prep["calls"])
    in_maps = build_in_maps(prep, W1, b1, W2, b2, Wf, bf)
    res = run_bass_kernel_spmd(nc, in_maps, core_ids=list(range(NCORES)))
    return np.asarray(res.results[0]["y"], np.float32)
